# revision 4
# baseline (speedup 1.0000x reference)
"""EME loss kernel for Trainium2, 8 NeuronCores, pure data-parallel.

Math (matches the jax reference):
  y_pred [32, 3, 1024, 1024] f32; 8x8 non-overlapping window max/min pooling;
  vals = 20*ln(max/(min+1e-4)); per_batch = sum(vals)/(1024*1024)*64;
  out = mean(per_batch) -> f32 scalar.

Sharding: batch across 8 cores (4 batches = 12 images of 1024x1024 per core).
Device computes per-partition partial sums of (ln(max) - ln(min+eps)); host
combines: out = total * 20 * 64 / 2^20 / 32.

Staging: input cast to bf16 on the host (round-to-nearest via ml_dtypes),
halving HBM traffic vs fp32 -- the kernel computed in bf16 anyway and the
rel-err budget is 2e-2 (measured ~8e-6 for bf16). Loads use plain HWDGE
(sync-engine) DMA; 24 MiB/core at ~358 GB/s/NC = ~70 us.

Layout: a 1024x1024 image viewed as [128, 8192] puts one window-row
(8 image rows, 16KB bf16 contiguous) on each partition; free idx =
r*1024 + w*8 + j (r=row-in-window, w=window, j=col-in-window).

Compute: pairwise max/min trees on DVE (bf16 tensor_tensor runs 2x =
2 elem/cycle/partition); DVE is the bottleneck (~100 us busy). The only
other engine that can help is GpSimd (Pool): its codegen supports only
add/subtract TT, so a slice of the min level-1 is offloaded there as
min(a,b) = (a+b) - max(a,b) with the sum in f32 (exact: bf16 inputs sum
exactly in f32, and min is itself a bf16 value). _POOL_E columns (of
2048) of each level-1 block go to Pool; 0 disables the offload.
"""
import numpy as np
import concourse.bass as bass
import concourse.mybir as mybir
import concourse.tile as tile
from concourse.bass_utils import run_bass_kernel_spmd

_N_CORES = 8
_B, _C, _H, _W = 32, 3, 1024, 1024
_IMGS_PER_CORE = (_B // _N_CORES) * _C  # 12
_PAIRS = _IMGS_PER_CORE // 2  # 6 (2 images per compute tile)
_WIN = 8
_EPS = 1e-4
_POOL_E = 896  # columns (of 2048) of each min-L1 block computed on Pool

_NC_CACHE = {}
LAST_RESULTS = None  # BassKernelResults of the most recent run (for test.py)


def _split_excess_waits(nc, max_waits=1):
    """This walrus build rejects >2 sync-waits on one CTRL instruction (the
    Tile exit drain collects one wait per active logical proc). Move excess
    waits onto preceding NoOps on the same engine."""
    for func in nc.m.functions:
        for bb in func.blocks:
            insts = bb.instructions
            out_insts = []
            changed = False
            for ins in insts:
                si = getattr(ins, "sync_info", None)
                if si is not None and si.on_wait and len(si.on_wait) > max_waits:
                    waits = list(si.on_wait)
                    head, tail = waits[:-max_waits], waits[-max_waits:]
                    for j in range(0, len(head), max_waits):
                        nop = mybir.InstNoOp(name=f"{ins.name}-wsplit{j}", ins=[], outs=[])
                        nop.engine = ins.engine
                        nop.sync_info = mybir.SyncInfo(
                            on_wait=head[j:j + max_waits], on_update=[])
                        out_insts.append(nop)
                    ins.sync_info = mybir.SyncInfo(on_wait=tail, on_update=si.on_update)
                    changed = True
                out_insts.append(ins)
            if changed:
                bb.instructions = out_insts


def _light_drain_and_barrier(self, tick_clock, wait_clock):
    """TileContext exit ceremony minus the trailing all-engine barrier
    (drain already waits on the global clock; NEFF completion waits on all
    engine programs regardless). Saves a few us of kernel-exit time."""
    from concourse.vector_clock import ScopedClock
    drain_inst = self.nc.sync.drain()
    wait_clock.add_sem_waits(drain_inst.ins,
                             ScopedClock({None: tick_clock.global_clock}))
    self.nc.all_engine_barrier()
    popped = self.nc._tile_sem_poison_stack.pop()
    assert popped is self._sem_poison
    self.nc._state.prepend_free_semaphores(
        [s.num if hasattr(s, "num") else s for s in self.sems.allocated().values()])


def _build():
    F32 = mybir.dt.float32
    BF16 = mybir.dt.bfloat16
    nc = bass.Bass()
    eps_t = nc.alloc_sbuf_tensor(f"const-float32-{_EPS}", [128, 1], F32)
    nc.gpsimd.memset(eps_t.ap(), _EPS)
    nc.const_aps.aps[(F32, _EPS)] = eps_t.ap()
    nc.all_engine_barrier()
    y = nc.declare_dram_parameter("y", [_IMGS_PER_CORE, _H, _W], BF16,
                                  isOutput=False)
    out = nc.declare_dram_parameter("out", [1, 1], F32, isOutput=True)

    MAX, MIN = mybir.AluOpType.max, mybir.AluOpType.min
    PE = _POOL_E

    tile.TileContext._drain_and_barrier = _light_drain_and_barrier
    with tile.TileContext(nc) as tc:
        with tc.tile_pool(name="img", bufs=2) as img_pool, \
             tc.tile_pool(name="l1", bufs=2) as l1_pool, \
             tc.tile_pool(name="tv", bufs=1) as tv_pool, \
             tc.tile_pool(name="tx", bufs=2) as tx_pool, \
             tc.tile_pool(name="stat", bufs=2) as stat_pool, \
             tc.tile_pool(name="accp", bufs=1) as acc_pool, \
             tc.tile_pool(name="psum", bufs=1, space="PSUM") as psum_pool:
            partsP = acc_pool.tile([128, _PAIRS], F32, tag="partsP")
            partsN = acc_pool.tile([128, _PAIRS], F32, tag="partsN")
            warm = acc_pool.tile([1, 1], BF16, tag="warm")
            nc.sync.dma_start(out=warm[:], in_=y[0, 0:1, 0:1])

            def htree(which, cur, op):
                """Horizontal j=8->1 on [128, 2048] (= [i, w, j8]) -> [128, 256]."""
                src, width = cur, 2048
                for jj in (4, 2, 1):
                    v = src[:].rearrange("p (i w k) -> p i w k", i=2, k=2 * jj)
                    pool = tx_pool if jj == 1 else tv_pool
                    nxt = pool.tile([128, width // 2], BF16, tag=f"{which}h{jj}")
                    nv = nxt[:].rearrange("p (i w k) -> p i w k", i=2, k=jj)
                    nc.vector.tensor_tensor(out=nv, in0=v[:, :, :, 0:jj],
                                            in1=v[:, :, :, jj:2 * jj], op=op)
                    src, width = nxt, width // 2
                return src  # [128, 256] = (i, w)

            for k in range(_PAIRS):
                T = img_pool.tile([128, 16384], BF16, tag="img")
                for i in range(2):
                    src = y[2 * k + i].rearrange("(p r) c -> p (r c)", p=128)
                    nc.sync.dma_start(out=T[:, i * 8192:i * 8192 + 4096],
                                      in_=src[:, 0:4096])
                for i in range(2):
                    src = y[2 * k + i].rearrange("(p r) c -> p (r c)", p=128)
                    nc.sync.dma_start(out=T[:, i * 8192 + 4096:(i + 1) * 8192],
                                      in_=src[:, 4096:8192])
                # W: [p, i(img), g(top/bot), h(row-pair), e]  e = r-parity*1024+w*8+j
                W = T[:].rearrange("p (i g h e) -> p i g h e", i=2, g=2, h=2, e=2048)
                mxa = l1_pool.tile([128, 8192], BF16, tag="mxa")  # [i, g, e]
                mna = l1_pool.tile([128, 8192], BF16, tag="mna")
                mxav = mxa[:].rearrange("p (i g e) -> p i g e", i=2, g=2)
                mnav = mna[:].rearrange("p (i g e) -> p i g e", i=2, g=2)
                for g in range(2):  # top rows (0-3) after top DMAs; bottom after
                    nc.vector.tensor_tensor(out=mxav[:, :, g, :],
                                            in0=W[:, :, g, 0, :],
                                            in1=W[:, :, g, 1, :], op=MAX)
                    if PE < 2048:
                        nc.vector.tensor_tensor(out=mnav[:, :, g, PE:2048],
                                                in0=W[:, :, g, 0, PE:2048],
                                                in1=W[:, :, g, 1, PE:2048], op=MIN)
                    if PE > 0:
                        # Pool: min(a,b) = (a+b) - max(a,b); sum exact in f32
                        S = tx_pool.tile([128, 2 * PE], F32, tag="S")
                        Sv = S[:].rearrange("p (i e) -> p i e", i=2)
                        nc.gpsimd.tensor_tensor(out=Sv, in0=W[:, :, g, 0, 0:PE],
                                                in1=W[:, :, g, 1, 0:PE],
                                                op=mybir.AluOpType.add)
                        nc.gpsimd.tensor_tensor(out=mnav[:, :, g, 0:PE], in0=Sv,
                                                in1=mxav[:, :, g, 0:PE],
                                                op=mybir.AluOpType.subtract)
                trees = []
                for which, a, op in (("mx", mxa, MAX), ("mn", mna, MIN)):
                    av = a[:].rearrange("p (i g e) -> p i g e", i=2, g=2)
                    c = tv_pool.tile([128, 4096], BF16, tag=f"{which}c")
                    cv = c[:].rearrange("p (i e) -> p i e", i=2)
                    nc.vector.tensor_tensor(out=cv, in0=av[:, :, 0, :],
                                            in1=av[:, :, 1, :], op=op)
                    cc = c[:].rearrange("p (i h e) -> p i h e", i=2, h=2)
                    cur = tv_pool.tile([128, 2048], BF16, tag=f"{which}v")
                    nc.vector.tensor_tensor(
                        out=cur[:].rearrange("p (i e) -> p i e", i=2),
                        in0=cc[:, :, 0, :], in1=cc[:, :, 1, :], op=op)
                    trees.append(htree(which, cur, op))
                mx, mn = trees
                lmx = stat_pool.tile([128, 256], F32, tag="lmx")
                lmn = stat_pool.tile([128, 256], F32, tag="lmn")
                nc.scalar.activation(lmx[:], mx[:], mybir.ActivationFunctionType.Ln,
                                     accum_out=partsP[:, k:k + 1])
                nc.scalar.activation(lmn[:], mn[:], mybir.ActivationFunctionType.Ln,
                                     bias=_EPS, accum_out=partsN[:, k:k + 1])
            aP = acc_pool.tile([128, 1], F32, tag="aP")
            nc.vector.tensor_reduce(out=aP[:], in_=partsP[:],
                                    axis=mybir.AxisListType.X,
                                    op=mybir.AluOpType.add)
            aN = acc_pool.tile([128, 1], F32, tag="aN")
            nc.vector.tensor_reduce(out=aN[:], in_=partsN[:],
                                    axis=mybir.AxisListType.X,
                                    op=mybir.AluOpType.add)
            acc = acc_pool.tile([128, 1], F32, tag="acc")
            nc.vector.tensor_tensor(out=acc[:], in0=aP[:], in1=aN[:],
                                    op=mybir.AluOpType.subtract)
            ones = nc.const_aps.tensor(1.0, (128, 1))
            pt = psum_pool.tile([1, 1], F32, tag="pt")
            nc.tensor.matmul(pt[:], acc[:], ones)
            total = acc_pool.tile([1, 1], F32, tag="total")
            nc.vector.tensor_copy(out=total[:], in_=pt[:])
            nc.sync.dma_start(out=out[:], in_=total[:])

    _split_excess_waits(nc)
    return nc


def _get_nc():
    if "nc" not in _NC_CACHE:
        _NC_CACHE["nc"] = _build()
    return _NC_CACHE["nc"]


def kernel(y_pred, winSize=8, _trace=False, **_ignored):
    global LAST_RESULTS
    assert int(winSize) == _WIN
    bf16 = mybir.dt.np(mybir.dt.bfloat16)
    y = np.ascontiguousarray(np.asarray(y_pred, dtype=np.float32)).astype(bf16)
    assert y.shape == (_B, _C, _H, _W)
    per_core_b = _B // _N_CORES
    in_maps = [
        {"y": y[c * per_core_b:(c + 1) * per_core_b].reshape(_IMGS_PER_CORE, _H, _W)}
        for c in range(_N_CORES)
    ]
    nc = _get_nc()
    res = run_bass_kernel_spmd(nc, in_maps, list(range(_N_CORES)), trace=_trace)
    LAST_RESULTS = res
    total = np.sum([float(r["out"][0, 0]) for r in res.results])
    val = total * 20.0 * (_WIN * _WIN) / (_H * _W) / _B
    return np.float32(val)


# revision 5
# speedup vs baseline: 1.5600x; 1.5600x over previous
"""EME loss kernel for Trainium2, 8 NeuronCores, pure data-parallel.

Math (matches the jax reference):
  y_pred [32, 3, 1024, 1024] f32; 8x8 non-overlapping window max/min pooling;
  vals = 20*ln(max/(min+1e-4)); per_batch = sum(vals)/(1024*1024)*64;
  out = mean(per_batch) -> f32 scalar.

Sharding: batch across 8 cores (4 batches = 12 images of 1024x1024 per core).
Device computes per-partition partial sums of (ln(max) - ln(min+eps)); host
combines: out = total * 20 * 64 / 2^20 / 32.

Staging: input cast to bf16 on the host (round-to-nearest via ml_dtypes),
halving HBM traffic vs fp32 -- the kernel computed in bf16 anyway and the
rel-err budget is 2e-2 (measured ~8e-6 for bf16). Loads use plain HWDGE
(sync-engine) DMA; 24 MiB/core at ~358 GB/s/NC = ~70 us.

Layout: a 1024x1024 image viewed as [128, 8192] puts one window-row
(8 image rows, 16KB bf16 contiguous) on each partition; free idx =
r*1024 + w*8 + j (r=row-in-window, w=window, j=col-in-window).

Compute: pairwise max/min trees on DVE (bf16 tensor_tensor runs 2x =
2 elem/cycle/partition); DVE is the bottleneck (~100 us busy). The only
other engine that can help is GpSimd (Pool): its codegen supports only
add/subtract TT, so a slice of the min level-1 is offloaded there as
min(a,b) = (a+b) - max(a,b) with the sum in f32 (exact: bf16 inputs sum
exactly in f32, and min is itself a bf16 value). _POOL_E columns (of
2048) of each level-1 block go to Pool; 0 disables the offload.
"""
import numpy as np
import concourse.bass as bass
import concourse.mybir as mybir
import concourse.tile as tile
from concourse.bass_utils import run_bass_kernel_spmd

_N_CORES = 8
_B, _C, _H, _W = 32, 3, 1024, 1024
_IMGS_PER_CORE = (_B // _N_CORES) * _C  # 12
_PAIRS = _IMGS_PER_CORE // 2  # 6 (2 images per compute tile)
_WIN = 8
_EPS = 1e-4
_POOL_E = 0  # columns (of 2048) of each min-L1 block computed on Pool.
# Measured: Pool TT add/sub runs ~1.4-1.8 ns/elem AND its SBUF traffic
# throttles concurrent DVE 2x-mode ops up to 2.4x -- the offload is a
# strict loss (202us vs 120us without). Keep 0.

_NC_CACHE = {}
LAST_RESULTS = None  # BassKernelResults of the most recent run (for test.py)


def _split_excess_waits(nc, max_waits=1):
    """This walrus build rejects >2 sync-waits on one CTRL instruction (the
    Tile exit drain collects one wait per active logical proc). Move excess
    waits onto preceding NoOps on the same engine."""
    for func in nc.m.functions:
        for bb in func.blocks:
            insts = bb.instructions
            out_insts = []
            changed = False
            for ins in insts:
                si = getattr(ins, "sync_info", None)
                if si is not None and si.on_wait and len(si.on_wait) > max_waits:
                    waits = list(si.on_wait)
                    head, tail = waits[:-max_waits], waits[-max_waits:]
                    for j in range(0, len(head), max_waits):
                        nop = mybir.InstNoOp(name=f"{ins.name}-wsplit{j}", ins=[], outs=[])
                        nop.engine = ins.engine
                        nop.sync_info = mybir.SyncInfo(
                            on_wait=head[j:j + max_waits], on_update=[])
                        out_insts.append(nop)
                    ins.sync_info = mybir.SyncInfo(on_wait=tail, on_update=si.on_update)
                    changed = True
                out_insts.append(ins)
            if changed:
                bb.instructions = out_insts


def _light_drain_and_barrier(self, tick_clock, wait_clock):
    """TileContext exit ceremony minus the trailing all-engine barrier
    (drain already waits on the global clock; NEFF completion waits on all
    engine programs regardless). Saves a few us of kernel-exit time."""
    from concourse.vector_clock import ScopedClock
    drain_inst = self.nc.sync.drain()
    wait_clock.add_sem_waits(drain_inst.ins,
                             ScopedClock({None: tick_clock.global_clock}))
    self.nc.all_engine_barrier()
    popped = self.nc._tile_sem_poison_stack.pop()
    assert popped is self._sem_poison
    self.nc._state.prepend_free_semaphores(
        [s.num if hasattr(s, "num") else s for s in self.sems.allocated().values()])


def _build():
    F32 = mybir.dt.float32
    BF16 = mybir.dt.bfloat16
    nc = bass.Bass()
    eps_t = nc.alloc_sbuf_tensor(f"const-float32-{_EPS}", [128, 1], F32)
    nc.gpsimd.memset(eps_t.ap(), _EPS)
    nc.const_aps.aps[(F32, _EPS)] = eps_t.ap()
    nc.all_engine_barrier()
    y = nc.declare_dram_parameter("y", [_IMGS_PER_CORE, _H, _W], BF16,
                                  isOutput=False)
    out = nc.declare_dram_parameter("out", [1, 1], F32, isOutput=True)

    MAX, MIN = mybir.AluOpType.max, mybir.AluOpType.min
    PE = _POOL_E

    tile.TileContext._drain_and_barrier = _light_drain_and_barrier
    with tile.TileContext(nc) as tc:
        with tc.tile_pool(name="img", bufs=2) as img_pool, \
             tc.tile_pool(name="l1", bufs=2) as l1_pool, \
             tc.tile_pool(name="tv", bufs=1) as tv_pool, \
             tc.tile_pool(name="tx", bufs=2) as tx_pool, \
             tc.tile_pool(name="stat", bufs=2) as stat_pool, \
             tc.tile_pool(name="accp", bufs=1) as acc_pool, \
             tc.tile_pool(name="psum", bufs=1, space="PSUM") as psum_pool:
            partsP = acc_pool.tile([128, _PAIRS], F32, tag="partsP")
            partsN = acc_pool.tile([128, _PAIRS], F32, tag="partsN")
            warm = acc_pool.tile([1, 1], BF16, tag="warm")
            nc.sync.dma_start(out=warm[:], in_=y[0, 0:1, 0:1])

            def htree(which, cur, op):
                """Horizontal j=8->1 on [128, 2048] (= [i, w, j8]) -> [128, 256]."""
                src, width = cur, 2048
                for jj in (4, 2, 1):
                    v = src[:].rearrange("p (i w k) -> p i w k", i=2, k=2 * jj)
                    pool = tx_pool if jj == 1 else tv_pool
                    nxt = pool.tile([128, width // 2], BF16, tag=f"{which}h{jj}")
                    nv = nxt[:].rearrange("p (i w k) -> p i w k", i=2, k=jj)
                    nc.vector.tensor_tensor(out=nv, in0=v[:, :, :, 0:jj],
                                            in1=v[:, :, :, jj:2 * jj], op=op)
                    src, width = nxt, width // 2
                return src  # [128, 256] = (i, w)

            for k in range(_PAIRS):
                T = img_pool.tile([128, 16384], BF16, tag="img")
                for i in range(2):
                    src = y[2 * k + i].rearrange("(p r) c -> p (r c)", p=128)
                    nc.sync.dma_start(out=T[:, i * 8192:i * 8192 + 4096],
                                      in_=src[:, 0:4096])
                for i in range(2):
                    src = y[2 * k + i].rearrange("(p r) c -> p (r c)", p=128)
                    nc.sync.dma_start(out=T[:, i * 8192 + 4096:(i + 1) * 8192],
                                      in_=src[:, 4096:8192])
                # W: [p, i(img), g(top/bot), h(row-pair), e]  e = r-parity*1024+w*8+j
                W = T[:].rearrange("p (i g h e) -> p i g h e", i=2, g=2, h=2, e=2048)
                mxa = l1_pool.tile([128, 8192], BF16, tag="mxa")  # [i, g, e]
                mna = l1_pool.tile([128, 8192], BF16, tag="mna")
                mxav = mxa[:].rearrange("p (i g e) -> p i g e", i=2, g=2)
                mnav = mna[:].rearrange("p (i g e) -> p i g e", i=2, g=2)
                for g in range(2):  # top rows (0-3) after top DMAs; bottom after
                    nc.vector.tensor_tensor(out=mxav[:, :, g, :],
                                            in0=W[:, :, g, 0, :],
                                            in1=W[:, :, g, 1, :], op=MAX)
                    if PE < 2048:
                        nc.vector.tensor_tensor(out=mnav[:, :, g, PE:2048],
                                                in0=W[:, :, g, 0, PE:2048],
                                                in1=W[:, :, g, 1, PE:2048], op=MIN)
                    if PE > 0:
                        # Pool: min(a,b) = (a+b) - max(a,b); sum exact in f32
                        S = tx_pool.tile([128, 2 * PE], F32, tag="S")
                        Sv = S[:].rearrange("p (i e) -> p i e", i=2)
                        nc.gpsimd.tensor_tensor(out=Sv, in0=W[:, :, g, 0, 0:PE],
                                                in1=W[:, :, g, 1, 0:PE],
                                                op=mybir.AluOpType.add)
                        nc.gpsimd.tensor_tensor(out=mnav[:, :, g, 0:PE], in0=Sv,
                                                in1=mxav[:, :, g, 0:PE],
                                                op=mybir.AluOpType.subtract)
                trees = []
                for which, a, op in (("mx", mxa, MAX), ("mn", mna, MIN)):
                    av = a[:].rearrange("p (i g e) -> p i g e", i=2, g=2)
                    c = tv_pool.tile([128, 4096], BF16, tag=f"{which}c")
                    cv = c[:].rearrange("p (i e) -> p i e", i=2)
                    nc.vector.tensor_tensor(out=cv, in0=av[:, :, 0, :],
                                            in1=av[:, :, 1, :], op=op)
                    cc = c[:].rearrange("p (i h e) -> p i h e", i=2, h=2)
                    cur = tv_pool.tile([128, 2048], BF16, tag=f"{which}v")
                    nc.vector.tensor_tensor(
                        out=cur[:].rearrange("p (i e) -> p i e", i=2),
                        in0=cc[:, :, 0, :], in1=cc[:, :, 1, :], op=op)
                    trees.append(htree(which, cur, op))
                mx, mn = trees
                lmx = stat_pool.tile([128, 256], F32, tag="lmx")
                lmn = stat_pool.tile([128, 256], F32, tag="lmn")
                nc.scalar.activation(lmx[:], mx[:], mybir.ActivationFunctionType.Ln,
                                     accum_out=partsP[:, k:k + 1])
                nc.scalar.activation(lmn[:], mn[:], mybir.ActivationFunctionType.Ln,
                                     bias=_EPS, accum_out=partsN[:, k:k + 1])
            aP = acc_pool.tile([128, 1], F32, tag="aP")
            nc.vector.tensor_reduce(out=aP[:], in_=partsP[:],
                                    axis=mybir.AxisListType.X,
                                    op=mybir.AluOpType.add)
            aN = acc_pool.tile([128, 1], F32, tag="aN")
            nc.vector.tensor_reduce(out=aN[:], in_=partsN[:],
                                    axis=mybir.AxisListType.X,
                                    op=mybir.AluOpType.add)
            acc = acc_pool.tile([128, 1], F32, tag="acc")
            nc.vector.tensor_tensor(out=acc[:], in0=aP[:], in1=aN[:],
                                    op=mybir.AluOpType.subtract)
            ones = nc.const_aps.tensor(1.0, (128, 1))
            pt = psum_pool.tile([1, 1], F32, tag="pt")
            nc.tensor.matmul(pt[:], acc[:], ones)
            total = acc_pool.tile([1, 1], F32, tag="total")
            nc.vector.tensor_copy(out=total[:], in_=pt[:])
            nc.sync.dma_start(out=out[:], in_=total[:])

    _split_excess_waits(nc)
    return nc


def _get_nc():
    if "nc" not in _NC_CACHE:
        _NC_CACHE["nc"] = _build()
    return _NC_CACHE["nc"]


def kernel(y_pred, winSize=8, _trace=False, **_ignored):
    global LAST_RESULTS
    assert int(winSize) == _WIN
    bf16 = mybir.dt.np(mybir.dt.bfloat16)
    y = np.ascontiguousarray(np.asarray(y_pred, dtype=np.float32)).astype(bf16)
    assert y.shape == (_B, _C, _H, _W)
    per_core_b = _B // _N_CORES
    in_maps = [
        {"y": y[c * per_core_b:(c + 1) * per_core_b].reshape(_IMGS_PER_CORE, _H, _W)}
        for c in range(_N_CORES)
    ]
    nc = _get_nc()
    res = run_bass_kernel_spmd(nc, in_maps, list(range(_N_CORES)), trace=_trace)
    LAST_RESULTS = res
    total = np.sum([float(r["out"][0, 0]) for r in res.results])
    val = total * 20.0 * (_WIN * _WIN) / (_H * _W) / _B
    return np.float32(val)
